# revision 3
# baseline (speedup 1.0000x reference)
"""GAT layer (multi-head graph attention) on 8 TRN2 NeuronCores.

Strategy (per sharding hint): destination nodes are sharded across the 8
cores.  Each core:
  phase 1: computes the full projection table redundantly (bf16 GEMM
           X @ W.T plus the per-head attention score reductions), packed
           as [proj bf16 | s_src f32 | s_tgt f32 | pad] rows in local HBM.
  phase 2: walks its shard's destination windows (128 targets / window).
           Edges are pre-sorted by (window, src-bucket) on the host;
           dma_gather pulls the source rows (int16 indices per 32768-row
           bucket), scores -> leaky-relu -> exp run batched per window,
           and one-hot matmuls (host-streamed) accumulate both the
           softmax denominator and the weighted aggregation in PSUM.
           Softmax division + PReLU happen once per window at flush.

kernel(**inputs) takes the FULL inputs and returns the FULL output.
"""

import math
from dataclasses import dataclass, field

import numpy as np
import ml_dtypes

BF16 = ml_dtypes.bfloat16
P = 128


def _ceil(a, b):
    return -(-a // b)


@dataclass
class Cfg:
    N: int = 100000
    E: int = 800000
    HID: int = 512
    HEADS: int = 8
    ncores: int = 8
    bucket: int = 32768
    leak: float = 0.01
    oh_bf16: bool = True  # one-hot stream dtype (bf16; fp8 is an option)

    def __post_init__(self):
        assert self.N % self.ncores == 0
        assert self.bucket <= 32768
        self.F = self.HID // self.HEADS
        self.shard = self.N // self.ncores
        self.NW = _ceil(self.shard, P)          # windows per core
        self.NB = _ceil(self.N, self.bucket)    # src buckets (int16 range)
        self.NT = _ceil(self.N, P)              # projection tiles
        self.NPAD = self.NT * P
        self.KP = min(self.HID, P)              # contraction partitions
        self.KT = self.HID // self.KP           # contraction tiles
        row_bytes = self.HID * 2 + 2 * self.HEADS * 4
        self.row_bytes = _ceil(row_bytes, 256) * 256
        self.row_bf = self.row_bytes // 2
        self.row_f32 = self.row_bytes // 4
        self.s_src_off = self.HID // 2          # f32 col of s_src in a row
        self.s_tgt_off = self.HID // 2 + self.HEADS


@dataclass
class Schedule:
    """Core-independent (uniform) phase-2 schedule."""
    seg: np.ndarray          # [NW, NB] slot counts (128-aligned, global max)
    TW: list                 # tiles per window
    TWmax: int
    calls: list              # per window: list of (b, slot_off, nslots, idxcol0)
    idxcols: int             # total int16 idx columns (per 16-wrap row)
    TT: int                  # total tiles
    tile_base: list          # first global tile index of each window


def build_schedule(cfg: Cfg, counts: np.ndarray) -> Schedule:
    """counts: [ncores, NW, NB] edge counts."""
    maxcnt = counts.max(axis=0)  # [NW, NB]
    seg = np.where(maxcnt > 0, _ceil(maxcnt, P) * P, 0).astype(np.int64)
    TW, calls, tile_base = [], [], []
    idxcol = 0
    tt = 0
    for w in range(cfg.NW):
        tile_base.append(tt)
        wcalls = []
        off = 0
        for b in range(cfg.NB):
            s = int(seg[w, b])
            if s == 0:
                continue
            wcalls.append((b, off, s, idxcol))
            off += s
            idxcol += s // 16
        assert off % P == 0
        TW.append(off // P)
        tt += off // P
        calls.append(wcalls)
    return Schedule(seg=seg, TW=TW, TWmax=max(TW), calls=calls,
                    idxcols=idxcol, TT=tt, tile_base=tile_base)


def prep_core(cfg: Cfg, sched: Schedule, src, trg, k):
    """Per-core input arrays: g1 idx stream and one-hot stream."""
    oh_dt = BF16 if cfg.oh_bf16 else ml_dtypes.float8_e4m3
    mask = (trg // cfg.shard) == k
    esrc = src[mask]
    etrg = trg[mask]
    trel = etrg - k * cfg.shard
    win = trel // P
    buck = esrc // cfg.bucket
    # order edges by (window, bucket); stable so host/device agree
    order = np.lexsort((buck, win))
    esrc, etrg, trel, win, buck = (a[order] for a in (esrc, etrg, trel, win, buck))

    g1i = np.zeros((P, sched.idxcols), np.int16)
    oh = np.zeros((P, sched.TT, 2, P), oh_dt)

    # per (window, bucket) segment boundaries
    key = win * cfg.NB + buck
    # edge ranges per (w, b)
    starts = np.searchsorted(key, np.arange(cfg.NW * cfg.NB), side="left")
    ends = np.searchsorted(key, np.arange(cfg.NW * cfg.NB), side="right")

    for w in range(cfg.NW):
        for (b, slot_off, nslots, idxcol0) in sched.calls[w]:
            lo, hi = int(starts[w * cfg.NB + b]), int(ends[w * cfg.NB + b])
            cnt = hi - lo
            assert cnt <= nslots
            idx = np.zeros(nslots, np.int16)
            idx[:cnt] = (esrc[lo:hi] - b * cfg.bucket).astype(np.int16)
            blk = idx.reshape(nslots // 16, 16).T          # [16, cols]
            g1i[:, idxcol0:idxcol0 + nslots // 16] = np.tile(blk, (8, 1))
            # one-hots for this segment's tiles
            tloc = (trel[lo:hi] - w * P).astype(np.int64)  # [cnt] in [0,128)
            t0 = sched.tile_base[w] + slot_off // P
            for j in range(nslots // P):
                s0, s1 = j * P, min((j + 1) * P, cnt)
                if s1 <= s0:
                    continue
                rows = np.arange(s0, s1) - s0
                cols = tloc[s0:s1]
                oh[rows, t0 + j, 0, cols] = oh_dt(1.0)
                oh[cols, t0 + j, 1, rows] = oh_dt(1.0)
    return g1i, oh


def pack_xt(cfg: Cfg, X: np.ndarray) -> np.ndarray:
    """X [N, HID] f32 -> bf16 packed [KP, NT, KT, P]: (p, j, ki, n) = X[j*P+n, ki*KP+p]."""
    Xp = np.zeros((cfg.NPAD, cfg.HID), np.float32)
    Xp[: cfg.N] = X
    Xb = Xp.astype(BF16)
    # [NT, P(n), KT, KP(p)] -> transpose to [KP, NT, KT, P]
    v = Xb.reshape(cfg.NT, P, cfg.KT, cfg.KP)
    return np.ascontiguousarray(v.transpose(3, 0, 2, 1))


def pack_w(cfg: Cfg, W, a_src, a_tgt):
    """Returns wt [KP, KT, HID] bf16 and wa [KP, KT, 2*HEADS] bf16."""
    WT = W.T.astype(np.float32)                       # [HID(d), HID(o)]
    wa_s = (W.reshape(cfg.HEADS, cfg.F, cfg.HID)
            * np.asarray(a_src, np.float32).reshape(cfg.HEADS, cfg.F, 1)).sum(1)  # [H, d]
    wa_t = (W.reshape(cfg.HEADS, cfg.F, cfg.HID)
            * np.asarray(a_tgt, np.float32).reshape(cfg.HEADS, cfg.F, 1)).sum(1)
    WA = np.concatenate([wa_s.T, wa_t.T], axis=1)     # [d, 2H]
    wt = np.ascontiguousarray(
        WT.astype(BF16).reshape(cfg.KT, cfg.KP, cfg.HID).transpose(1, 0, 2))
    wa = np.ascontiguousarray(
        WA.astype(BF16).reshape(cfg.KT, cfg.KP, 2 * cfg.HEADS).transpose(1, 0, 2))
    return wt, wa


def _bcast_last(ap, n):
    """Append a 0-stride broadcast dim of size n to an AP."""
    import concourse.bass as bass
    lst = [list(x) for x in ap.ap] + [[0, n]]
    return bass.AP(ap.tensor, ap.offset, lst)


def build_nc(cfg: Cfg, sched: Schedule):
    import concourse.bacc as bacc
    import concourse.bass as bass
    import concourse.mybir as mybir
    from concourse.tile import TileContext

    dt = mybir.dt
    oh_mdt = dt.bfloat16 if cfg.oh_bf16 else dt.float8e4
    H, HID, KT, KP = cfg.HEADS, cfg.HID, cfg.KT, cfg.KP

    nc = bacc.Bacc("TRN2", target_bir_lowering=False)

    xt = nc.dram_tensor("xt", [KP, cfg.NT, KT, P], dt.bfloat16, kind="ExternalInput")
    wt = nc.dram_tensor("wt", [KP, KT, HID], dt.bfloat16, kind="ExternalInput")
    wa = nc.dram_tensor("wa", [KP, KT, 2 * H], dt.bfloat16, kind="ExternalInput")
    g1i = nc.dram_tensor("g1i", [P, sched.idxcols], dt.int16, kind="ExternalInput")
    ohd = nc.dram_tensor("ohd", [P, sched.TT, 2, P], oh_mdt, kind="ExternalInput")
    avec = nc.dram_tensor("avec", [P, 1], dt.float32, kind="ExternalInput")
    out = nc.dram_tensor("out", [cfg.NW * P, HID], dt.float32, kind="ExternalOutput")

    with TileContext(nc) as tc:
        with tc.tile_pool(name="const", bufs=1) as cpool, \
             tc.tile_pool(name="dram", bufs=1, space="DRAM") as dpool:
            table = dpool.tile([cfg.NPAD, cfg.row_bf], dt.bfloat16)
            wt_sb = cpool.tile([KP, KT, HID], dt.bfloat16)
            nc.sync.dma_start(out=wt_sb[:], in_=wt[:, :, :])
            wa_sb = cpool.tile([KP, KT, 2 * H], dt.bfloat16)
            nc.sync.dma_start(out=wa_sb[:], in_=wa[:, :, :])
            a_sb = cpool.tile([P, 1], dt.float32)
            nc.sync.dma_start(out=a_sb[:], in_=avec[:, :])
            g1i_sb = cpool.tile([P, sched.idxcols], dt.int16)
            nc.sync.dma_start(out=g1i_sb[:], in_=g1i[:, :])

            # ---------------- phase 1: projection table ----------------
            with tc.tile_pool(name="p1", bufs=3) as xpool, \
                 tc.tile_pool(name="p1ps", bufs=2, space="PSUM") as pspool, \
                 tc.tile_pool(name="p1st", bufs=3) as stpool:
                for j in range(cfg.NT):
                    xtile = xpool.tile([KP, KT, P], dt.bfloat16, tag="x")
                    nc.sync.dma_start(out=xtile[:], in_=xt[:, j, :, :])
                    ps1 = pspool.tile([P, HID], dt.float32, tag="ps1")
                    ps2 = pspool.tile([P, 2 * H], dt.float32, tag="ps2")
                    for ki in range(KT):
                        nc.tensor.matmul(ps1[:], xtile[:, ki, :], wt_sb[:, ki, :],
                                         start=(ki == 0), stop=(ki == KT - 1))
                    for ki in range(KT):
                        nc.tensor.matmul(ps2[:], xtile[:, ki, :], wa_sb[:, ki, :],
                                         start=(ki == 0), stop=(ki == KT - 1))
                    stg = stpool.tile([P, cfg.row_bf], dt.bfloat16, tag="stg")
                    stg32 = stg.bitcast(dt.float32)
                    nc.scalar.copy(out=stg[:, 0:HID], in_=ps1[:])
                    nc.scalar.copy(out=stg32[:, cfg.s_src_off:cfg.s_src_off + 2 * H],
                                   in_=ps2[:])
                    if cfg.s_tgt_off + H < cfg.row_f32:
                        nc.vector.memset(stg32[:, cfg.s_tgt_off + H:cfg.row_f32], 0.0)
                    nc.sync.dma_start(out=table[j * P:(j + 1) * P, :], in_=stg[:])

            tc.strict_bb_all_engine_barrier()

            # ---------------- phase 1.5: resident s_tgt (hi/lo bf16) ----------------
            pid = nc.sync.partition_id()
            table32 = table.bitcast(dt.float32)
            s_ap = table32[bass.DynSlice(pid * cfg.shard, cfg.NW * P),
                           cfg.s_tgt_off:cfg.s_tgt_off + H]
            s_ap = s_ap.rearrange("(w p) h -> p w h", p=P)
            s_all = cpool.tile([P, cfg.NW, H], dt.float32)
            nc.sync.dma_start(out=s_all[:], in_=s_ap)
            s_hilo = cpool.tile([P, cfg.NW, 2, H], dt.bfloat16)
            s_hi32 = cpool.tile([P, cfg.NW, H], dt.float32)
            nc.vector.tensor_copy(out=s_hilo[:, :, 0, :], in_=s_all[:])
            nc.vector.tensor_copy(out=s_hi32[:], in_=s_hilo[:, :, 0, :])
            nc.vector.tensor_tensor(out=s_hilo[:, :, 1, :], in0=s_all[:],
                                    in1=s_hi32[:], op=mybir.AluOpType.subtract)

            # ---------------- phase 2: windows ----------------
            with tc.tile_pool(name="p2", bufs=2) as pool, \
                 tc.tile_pool(name="p2ps", bufs=2, space="PSUM") as pps:
                for w in range(cfg.NW):
                    Tw = sched.TW[w]
                    g1t = pool.tile([P, sched.TWmax, cfg.row_bf], dt.bfloat16, tag="g1t")
                    for (b, slot_off, nslots, idxcol0) in sched.calls[w]:
                        rows = min(cfg.NPAD, (b + 1) * cfg.bucket) - b * cfg.bucket
                        nc.gpsimd.dma_gather(
                            g1t[:, slot_off // P:(slot_off + nslots) // P, :],
                            table[b * cfg.bucket:b * cfg.bucket + rows, :],
                            g1i_sb[:, idxcol0:idxcol0 + nslots // 16],
                            nslots, nslots, cfg.row_bf)
                    jb = sched.tile_base[w]
                    oht = pool.tile([P, sched.TWmax, 2, P], oh_mdt, tag="oht")
                    nc.sync.dma_start(out=oht[:, :Tw, :, :], in_=ohd[:, jb:jb + Tw, :, :])

                    # s_tgt expansion (per tile) via transposed one-hot matmul
                    stgt = pps.tile([P, sched.TWmax, 2, H], dt.float32, tag="stgt")
                    for t in range(Tw):
                        nc.tensor.matmul(stgt[:, t, :, :], oht[:, t, 1, :],
                                         s_hilo[:, w, :, :], start=True, stop=True)
                    g1t32 = g1t.bitcast(dt.float32)
                    s_sum = pool.tile([P, sched.TWmax, H], dt.float32, tag="s_sum")
                    s_act = pool.tile([P, sched.TWmax, H], dt.float32, tag="s_act")
                    nc.vector.tensor_tensor(
                        out=s_sum[:, :Tw, :], in0=stgt[:, :Tw, 0, :],
                        in1=g1t32[:, :Tw, cfg.s_src_off:cfg.s_src_off + H],
                        op=mybir.AluOpType.add)
                    nc.vector.tensor_tensor(
                        out=s_act[:, :Tw, :], in0=stgt[:, :Tw, 1, :],
                        in1=s_sum[:, :Tw, :], op=mybir.AluOpType.add)
                    nc.vector.scalar_tensor_tensor(
                        out=s_sum[:, :Tw, :], in0=s_act[:, :Tw, :], scalar=cfg.leak,
                        in1=s_act[:, :Tw, :], op0=mybir.AluOpType.mult,
                        op1=mybir.AluOpType.max)
                    exp_t = pool.tile([P, sched.TWmax, H], dt.bfloat16, tag="exp_t")
                    nc.scalar.activation(out=exp_t[:, :Tw, :], in_=s_sum[:, :Tw, :],
                                         func=mybir.ActivationFunctionType.Exp)

                    w_t = pool.tile([P, sched.TWmax, HID], dt.bfloat16, tag="w_t")
                    proj4 = g1t[:, :Tw, 0:HID].rearrange("p t (h f) -> p t h f", h=H)
                    exp4 = _bcast_last(exp_t[:, :Tw, :], cfg.F)
                    out4 = w_t[:, :Tw, :].rearrange("p t (h f) -> p t h f", h=H)
                    nc.vector.tensor_tensor(out=out4, in0=proj4, in1=exp4,
                                            op=mybir.AluOpType.mult)

                    agg = pps.tile([P, HID], dt.float32, tag="agg")
                    den = pps.tile([P, H], dt.float32, tag="den")
                    for t in range(Tw):
                        nc.tensor.matmul(agg[:], oht[:, t, 0, :], w_t[:, t, :],
                                         start=(t == 0), stop=(t == Tw - 1))
                        nc.tensor.matmul(den[:], oht[:, t, 0, :], exp_t[:, t, :],
                                         start=(t == 0), stop=(t == Tw - 1))

                    # flush: softmax divide + PReLU
                    den_sb = pool.tile([P, H, 1], dt.float32, tag="den_sb")
                    recip = pool.tile([P, H, 1], dt.float32, tag="recip")
                    nc.vector.tensor_scalar_add(out=den_sb[:, :, 0], in0=den[:],
                                                scalar1=1e-16)
                    nc.vector.reciprocal(out=recip[:], in_=den_sb[:])
                    z = pool.tile([P, HID], dt.float32, tag="z")
                    agg4 = agg[:].rearrange("p (h f) -> p h f", h=H)
                    z4 = z[:].rearrange("p (h f) -> p h f", h=H)
                    nc.vector.tensor_tensor(out=z4, in0=agg4,
                                            in1=_bcast_last(recip[:, :, 0], cfg.F),
                                            op=mybir.AluOpType.mult)
                    res = pool.tile([P, HID], dt.float32, tag="res")
                    nc.vector.scalar_tensor_tensor(
                        out=res[:], in0=z[:], scalar=a_sb[:, 0:1], in1=z[:],
                        op0=mybir.AluOpType.mult, op1=mybir.AluOpType.max)
                    nc.sync.dma_start(out=out[w * P:(w + 1) * P, :], in_=res[:])

    nc.compile()
    return nc


def prepare(cfg: Cfg, inputs):
    """Host-side prep shared by HW and sim paths.

    Returns (sched, in_maps, assemble) where assemble(core_outs) -> full out.
    """
    X = np.asarray(inputs["in_nodes_features"], np.float32)
    ei = np.asarray(inputs["edge_index"], np.int64)
    W = np.asarray(inputs["W"], np.float32)
    b_lin = np.asarray(inputs["b_lin"], np.float32)
    a_src = np.asarray(inputs["a_src"], np.float32)
    a_tgt = np.asarray(inputs["a_tgt"], np.float32)
    bias = np.asarray(inputs["bias"], np.float32)
    prelu_a = float(np.asarray(inputs["prelu_a"], np.float32))

    assert np.all(b_lin == 0) and np.all(bias == 0), "nonzero bias unsupported"
    assert 0.0 <= prelu_a <= 1.0, "prelu_a outside [0,1] unsupported"

    src, trg = ei[0], ei[1]
    core_of = trg // cfg.shard
    win_of = (trg % cfg.shard) // P
    buck_of = src // cfg.bucket
    counts = np.zeros((cfg.ncores, cfg.NW, cfg.NB), np.int64)
    for k in range(cfg.ncores):
        m = core_of == k
        counts[k] = np.bincount(
            win_of[m] * cfg.NB + buck_of[m],
            minlength=cfg.NW * cfg.NB).reshape(cfg.NW, cfg.NB)
    sched = build_schedule(cfg, counts)

    xt = pack_xt(cfg, X)
    wtp, wap = pack_w(cfg, W, a_src, a_tgt)
    av = np.full((P, 1), prelu_a, np.float32)

    in_maps = []
    for k in range(cfg.ncores):
        g1i_k, oh_k = prep_core(cfg, sched, src, trg, k)
        in_maps.append({
            "xt": xt, "wt": wtp, "wa": wap,
            "g1i": g1i_k, "ohd": oh_k, "avec": av,
        })

    def assemble(core_outs):
        return np.concatenate(
            [np.asarray(o["out"][: cfg.shard], np.float32) for o in core_outs], axis=0)

    return sched, in_maps, assemble


_BUILT = {}


def _get_built(cfg: Cfg, sched: Schedule):
    key = (cfg.N, cfg.E, cfg.HID, cfg.HEADS, cfg.ncores, cfg.bucket,
           tuple(sched.TW), sched.idxcols)
    if key not in _BUILT:
        _BUILT[key] = build_nc(cfg, sched)
    return _BUILT[key]


def kernel(**inputs):
    from concourse.bass_utils import run_bass_kernel_spmd

    cfg = Cfg()
    sched, in_maps, assemble = prepare(cfg, inputs)
    nc = _get_built(cfg, sched)
    res = run_bass_kernel_spmd(nc, in_maps, core_ids=list(range(cfg.ncores)))
    return assemble(res.results)
